# revision 11
# baseline (speedup 1.0000x reference)
"""Weighted 2D cross-entropy (BCE-over-classes) loss on 8 Trainium2 cores.

Math (matches the reference):
  t in [0,19); pos = t>0, neg = t==0 (all pixels are pos or neg; mask == 1)
  S(i) = sum_c bce(i,c) = -[ B(i) + A(i) ]
     A(i) = sum_c log(1-p_c(i))
     B(i) = log(p_t(i)) - log(1-p_t(i))
  loss = ( (NEG/TOT)*S_pos_sum + (POS/TOT)*S_neg_sum ) / (TOT*C)

Per-core (core k <- batch element k, pure data parallel):
  - target is uploaded pre-converted to bf16 by the host (halves its DMA
    bytes and kills the on-device int->bf16 conversion); the pos count is
    computed host-side from the int32 labels.
  - per class-tile: ACT pass L_c = Ln(1-p_c) in bf16; DVE eq mask (T==c)
    and masked_c = eq*L_c; PE identity-matmuls accumulate A = sum_c L_c
    and L_sel = sum_c masked_c into PSUM f32 (the log(1-p) gather at the
    target class).
  - per-tile tail: B = Ln(exp(-L_sel) - 1) = log(p_t) - log(1-p_t); DVE
    reduces give sum A / sum B, STTs give their pos-masked sums.
  - the last tile splits its final class DMA and its tail into 512-col
    halves (phase-ordered exp,exp,ln,ln to pay the ACT function-table
    reload only twice), so the post-last-byte chain is short.
  - the whole [128, STAT_COLS] stats tile is DMA'd out; the host does all
    folding (partition+column sums and the cross-core "all-reduce") in f64.
"""

from contextlib import ExitStack

import numpy as np

import concourse.bass as bass
import concourse.mybir as mybir
import concourse.tile as tile
from concourse import bacc
from concourse.bass_utils import run_bass_kernel_spmd

# problem shape (hardcoded per harness contract)
N, C, H, W = 8, 19, 512, 1024
PIX = H * W          # 524288 pixels per core
P = 128              # partitions
FCOLS = PIX // P     # 4096 free columns when pixels laid out [128, 4096]
FT = 1024            # pixel-tile free width
NTILES = FCOLS // FT # 4 pixel tiles per core
HALF = FT // 2       # last tile's final class + tail split into 512-col halves
N_CORES = 8

DT = mybir.dt

# stats buffer column layout (all f32, each column written exactly once)
N_TAILS = (NTILES - 1) + 2              # 3 full tails + 2 half tails = 5
COL_UALL = 0                            # sum A per tail
COL_B = COL_UALL + N_TAILS              # sum B per tail
COL_POSB = COL_B + N_TAILS              # sum pos*B per tail
COL_POSA = COL_POSB + N_TAILS           # sum pos*A per tail
STAT_COLS = COL_POSA + N_TAILS          # 20
NSTAT = STAT_COLS  # legacy alias


def build_kernel() -> bass.Bass:
    # Bacc (not raw Bass): its compile() pipeline runs
    # generate_event_semaphores, which splits multi-sem waits to satisfy the
    # 1-wait-per-instruction TRN2 sync structs -- raw Bass modules with
    # Tile-emitted multi-waits fail walrus codegen.
    nc = bacc.Bacc("TRN2")

    predict = nc.declare_dram_parameter("predict", [C, PIX], DT.float32, isOutput=False)
    target = nc.declare_dram_parameter("target", [P, FCOLS], DT.bfloat16, isOutput=False)
    idn = nc.declare_dram_parameter("idn", [P, P], DT.bfloat16, isOutput=False)
    out = nc.declare_dram_parameter("out", [P, STAT_COLS], DT.float32, isOutput=True)

    pred_r = predict.rearrange("c (p f) -> c p f", p=P)  # [19, 128, 4096]

    with tile.TileContext(nc) as tc, ExitStack() as ctx:
        const = ctx.enter_context(tc.tile_pool(name="const", bufs=1))
        p_pool = ctx.enter_context(tc.tile_pool(name="p", bufs=24))
        lm_pool = ctx.enter_context(tc.tile_pool(name="lm", bufs=12))
        pix_pool = ctx.enter_context(tc.tile_pool(name="pix", bufs=2))
        scr_pool = ctx.enter_context(tc.tile_pool(name="scr", bufs=2))
        eq_pool = ctx.enter_context(tc.tile_pool(name="eq", bufs=4))
        psum_pool = ctx.enter_context(tc.tile_pool(name="ps", bufs=2, space="PSUM"))

        # constants + target go through the gpsimd queue so the sync queue's
        # first instruction is the first predict load
        idn_sb = const.tile([P, P], DT.bfloat16, tag="idn")
        nc.gpsimd.dma_start(out=idn_sb[:], in_=idn[:])
        t_bf = const.tile([P, FCOLS], DT.bfloat16, tag="tb")
        nc.gpsimd.dma_start(out=t_bf[:], in_=target[:])

        stats = const.tile([P, STAT_COLS], DT.float32, tag="stats")
        neg1 = const.tile([P, 1], DT.float32, tag="neg1")
        nc.gpsimd.memset(neg1[:], -1.0)
        two_ = const.tile([P, 1], DT.float32, tag="two")
        nc.gpsimd.memset(two_[:], 2.0)
        dscr = const.tile([P, 1], DT.float32, tag="dscr")

        def class_pass(p_src, t_sl, w, acc_ps, acc_off, c, last_c):
            """One class over w pixel cols: Ln, eq, mask, PE accumulate.

            lm layout [L(w) | masked(w)]; A contribs go to acc cols
            [acc_off, acc_off+w), Lsel contribs to [FT+acc_off, FT+acc_off+w).
            """
            lm_full = lm_pool.tile([P, 2 * FT], DT.bfloat16, tag="lm", name="lm")
            lm = lm_full[:, : 2 * w]
            nc.scalar.activation(
                out=lm[:, :w],
                in_=p_src,
                func=mybir.ActivationFunctionType.Ln,
                bias=1.0,
                scale=-1.0,
            )
            eq_full = eq_pool.tile([P, FT], DT.bfloat16, tag="eq", name="eq")
            eq = eq_full[:, :w]
            nc.vector.tensor_scalar(
                out=eq[:],
                in0=t_sl,
                scalar1=float(c),
                scalar2=None,
                op0=mybir.AluOpType.is_equal,
            )
            nc.vector.tensor_mul(out=lm[:, w:], in0=eq[:], in1=lm[:, :w])
            nseg = w // 512
            for s in range(2 * nseg):
                ssl = slice(s * 512, (s + 1) * 512)
                if s < nseg:
                    aoff = acc_off + s * 512
                else:
                    aoff = FT + acc_off + (s - nseg) * 512
                nc.tensor.matmul(
                    acc_ps[:, aoff : aoff + 512],
                    lhsT=idn_sb[:],
                    rhs=lm[:, ssl],
                    start=(c == 0),
                    stop=last_c,
                )

        for t in range(NTILES):
            fsl = slice(t * FT, (t + 1) * FT)
            last = t == NTILES - 1
            # PSUM acc: [A(1024) | L_sel(1024)]
            acc_ps = psum_pool.tile([P, 2 * FT], DT.float32, tag="acc")

            for c in range(C):
                p_t = p_pool.tile([P, FT], DT.float32, tag="p")
                if last and c == C - 1:
                    # split the final class: DMA + compute per 512-col half so
                    # the end-of-stream dependency chain operates on halves
                    for h in range(2):
                        hsl = slice(h * HALF, (h + 1) * HALF)
                        gsl = slice(t * FT + h * HALF, t * FT + (h + 1) * HALF)
                        nc.sync.dma_start(out=p_t[:, hsl], in_=pred_r[c, :, gsl])
                        class_pass(
                            p_t[:, hsl], t_bf[:, gsl], HALF, acc_ps, h * HALF, c, True
                        )
                else:
                    # p bufs=8 aligns slot reuse with the global DMA->DMAHW-
                    # proc round-robin (8 procs), so the WAW on the old writer
                    # is same-proc FIFO order and Tile emits no cross-queue wait
                    nc.sync.dma_start(out=p_t[:], in_=pred_r[c, :, fsl])
                    class_pass(p_t[:], t_bf[:, fsl], FT, acc_ps, 0, c, c == C - 1)

            if not last:
                # tail over the full 1024 cols
                k = t
                a_ps = acc_ps[:, :FT]
                lsel_ps = acc_ps[:, FT:]
                t_sl = t_bf[:, fsl]
                nc.vector.tensor_reduce(
                    out=stats[:, COL_UALL + k : COL_UALL + k + 1],
                    in_=a_ps,
                    axis=mybir.AxisListType.X,
                    op=mybir.AluOpType.add,
                )
                scr = scr_pool.tile([P, FT], DT.float32, tag="scr", name="scr")
                nc.vector.scalar_tensor_tensor(
                    out=scr[:],
                    in0=t_sl,
                    scalar=0.5,
                    in1=a_ps,
                    op0=mybir.AluOpType.is_gt,
                    op1=mybir.AluOpType.mult,
                    accum_out=stats[:, COL_POSA + k : COL_POSA + k + 1],
                )
                expl = pix_pool.tile([P, FT], DT.float32, tag="expl", name="expl")
                nc.scalar.activation(
                    out=expl[:],
                    in_=lsel_ps,
                    func=mybir.ActivationFunctionType.Exp,
                    scale=-1.0,
                )
                bq = pix_pool.tile([P, FT], DT.float32, tag="bq", name="bq")
                nc.scalar.activation(
                    out=bq[:],
                    in_=expl[:],
                    func=mybir.ActivationFunctionType.Ln,
                    bias=neg1[:, 0:1],
                )
                nc.vector.tensor_reduce(
                    out=stats[:, COL_B + k : COL_B + k + 1],
                    in_=bq[:],
                    axis=mybir.AxisListType.X,
                    op=mybir.AluOpType.add,
                )
                nc.vector.scalar_tensor_tensor(
                    out=scr[:],
                    in0=t_sl,
                    scalar=0.5,
                    in1=bq[:],
                    op0=mybir.AluOpType.is_gt,
                    op1=mybir.AluOpType.mult,
                    accum_out=stats[:, COL_POSB + k : COL_POSB + k + 1],
                )
            else:
                # last tile: tail in two 512-col halves, ACT phases shared so
                # the Exp/Ln function tables are each loaded once; a dummy
                # 1-col Exp right after the final class Ln pulls the Exp table
                # load off the critical chain (it has no data deps)
                hs = [
                    slice(t * FT + h * HALF, t * FT + (h + 1) * HALF) for h in range(2)
                ]
                a_h = [acc_ps[:, h * HALF : (h + 1) * HALF] for h in range(2)]
                lsel_h = [
                    acc_ps[:, FT + h * HALF : FT + (h + 1) * HALF] for h in range(2)
                ]
                expl_t = pix_pool.tile([P, FT], DT.float32, tag="expl", name="expl")
                bq_t = pix_pool.tile([P, FT], DT.float32, tag="bq", name="bq")
                scr = scr_pool.tile([P, FT], DT.float32, tag="scr", name="scr")
                for h in range(2):
                    nc.vector.tensor_reduce(
                        out=stats[:, COL_UALL + NTILES - 1 + h : COL_UALL + NTILES + h],
                        in_=a_h[h],
                        axis=mybir.AxisListType.X,
                        op=mybir.AluOpType.add,
                    )
                    nc.vector.scalar_tensor_tensor(
                        out=scr[:, h * HALF : (h + 1) * HALF],
                        in0=t_bf[:, hs[h]],
                        scalar=0.5,
                        in1=a_h[h],
                        op0=mybir.AluOpType.is_gt,
                        op1=mybir.AluOpType.mult,
                        accum_out=stats[
                            :, COL_POSA + NTILES - 1 + h : COL_POSA + NTILES + h
                        ],
                    )
                nc.scalar.activation(
                    out=dscr[:],
                    in_=two_[:],
                    func=mybir.ActivationFunctionType.Exp,
                    scale=-1.0,
                )
                for h in range(2):
                    nc.scalar.activation(
                        out=expl_t[:, h * HALF : (h + 1) * HALF],
                        in_=lsel_h[h],
                        func=mybir.ActivationFunctionType.Exp,
                        scale=-1.0,
                    )
                for h in range(2):
                    nc.scalar.activation(
                        out=bq_t[:, h * HALF : (h + 1) * HALF],
                        in_=expl_t[:, h * HALF : (h + 1) * HALF],
                        func=mybir.ActivationFunctionType.Ln,
                        bias=neg1[:, 0:1],
                    )
                for h in range(2):
                    nc.vector.tensor_reduce(
                        out=stats[:, COL_B + NTILES - 1 + h : COL_B + NTILES + h],
                        in_=bq_t[:, h * HALF : (h + 1) * HALF],
                        axis=mybir.AxisListType.X,
                        op=mybir.AluOpType.add,
                    )
                    nc.vector.scalar_tensor_tensor(
                        out=scr[:, h * HALF : (h + 1) * HALF],
                        in0=t_bf[:, hs[h]],
                        scalar=0.5,
                        in1=bq_t[:, h * HALF : (h + 1) * HALF],
                        op0=mybir.AluOpType.is_gt,
                        op1=mybir.AluOpType.mult,
                        accum_out=stats[
                            :, COL_POSB + NTILES - 1 + h : COL_POSB + NTILES + h
                        ],
                    )

        # ship the raw stats tile; the host folds partitions/columns in f64
        nc.sync.dma_start(out=out[:], in_=stats[:])

    if not nc.is_finalized():
        nc.finalize()

    return nc


def combine_stats(stats_list, pos_total: float) -> np.float32:
    """Host-side fold of the per-core [P, STAT_COLS] stats tiles (f64)."""
    u_all = b_sum = pos_b = pos_a = np.float64(0.0)
    for st in stats_list:
        st = np.asarray(st, dtype=np.float64).reshape(P, STAT_COLS)
        u_all += st[:, COL_UALL : COL_UALL + N_TAILS].sum()
        b_sum += st[:, COL_B : COL_B + N_TAILS].sum()
        pos_b += st[:, COL_POSB : COL_POSB + N_TAILS].sum()
        pos_a += st[:, COL_POSA : COL_POSA + N_TAILS].sum()
    tot = np.float64(len(stats_list) * PIX)
    s_all = -(b_sum + u_all)
    s_pos = -(pos_b + pos_a)
    pos = np.float64(pos_total)
    neg = tot - pos
    s_neg = s_all - s_pos
    loss = ((neg / tot) * s_pos + (pos / tot) * s_neg) / (tot * C)
    return np.float32(loss)


def host_pos(target: np.ndarray) -> float:
    return float((np.asarray(target) > 0).sum())


def make_in_maps(predict: np.ndarray, target: np.ndarray):
    import ml_dtypes

    predict = np.ascontiguousarray(predict, dtype=np.float32)
    target_bf = np.ascontiguousarray(
        np.asarray(target, dtype=np.int32).astype(ml_dtypes.bfloat16)
    )
    idn = np.eye(P, dtype=np.float32).astype(ml_dtypes.bfloat16)
    return [
        {
            "predict": predict[k].reshape(C, PIX),
            "target": target_bf[k].reshape(P, FCOLS),
            "idn": idn,
        }
        for k in range(N_CORES)
    ]


_NC_CACHE = None


def kernel(predict: np.ndarray, target: np.ndarray) -> np.ndarray:
    global _NC_CACHE
    if _NC_CACHE is None:
        _NC_CACHE = build_kernel()
    nc = _NC_CACHE

    in_maps = make_in_maps(predict, target)
    res = run_bass_kernel_spmd(nc, in_maps, list(range(N_CORES)))
    return combine_stats(
        [res.results[k]["out"] for k in range(N_CORES)], host_pos(target)
    )


# revision 12
# speedup vs baseline: 1.0537x; 1.0537x over previous
"""Weighted 2D cross-entropy (BCE-over-classes) loss on 8 Trainium2 cores.

Math (matches the reference):
  t in [0,19); pos = t>0, neg = t==0 (all pixels are pos or neg; mask == 1)
  S(i) = sum_c bce(i,c) = -[ B(i) + A(i) ]
     A(i) = sum_c log(1-p_c(i))
     B(i) = log(p_t(i)) - log(1-p_t(i))
  loss = ( (NEG/TOT)*S_pos_sum + (POS/TOT)*S_neg_sum ) / (TOT*C)

Per-core (core k <- batch element k, pure data parallel):
  - target is uploaded pre-converted to bf16 by the host (halves its DMA
    bytes and kills the on-device int->bf16 conversion); the pos count is
    computed host-side from the int32 labels.
  - per class-tile: ACT pass L_c = Ln(1-p_c) in bf16; DVE eq mask (T==c)
    and masked_c = eq*L_c; PE identity-matmuls accumulate A = sum_c L_c
    and L_sel = sum_c masked_c into PSUM f32 (the log(1-p) gather at the
    target class).
  - per-tile tail: B = Ln(exp(-L_sel) - 1) = log(p_t) - log(1-p_t); DVE
    reduces give sum A / sum B, STTs give their pos-masked sums.
  - the last tile splits its final class DMA and its tail into 512-col
    halves (phase-ordered exp,exp,ln,ln to pay the ACT function-table
    reload only twice), so the post-last-byte chain is short.
  - the whole [128, STAT_COLS] stats tile is DMA'd out; the host does all
    folding (partition+column sums and the cross-core "all-reduce") in f64.
"""

from contextlib import ExitStack

import numpy as np

import concourse.bass as bass
import concourse.mybir as mybir
import concourse.tile as tile
from concourse import bacc
from concourse.bass_utils import run_bass_kernel_spmd

# problem shape (hardcoded per harness contract)
N, C, H, W = 8, 19, 512, 1024
PIX = H * W          # 524288 pixels per core
P = 128              # partitions
FCOLS = PIX // P     # 4096 free columns when pixels laid out [128, 4096]
FT = 1024            # pixel-tile free width
NTILES = FCOLS // FT # 4 pixel tiles per core
HALF = FT // 2       # last tile's final class + tail split into 512-col halves
N_CORES = 8

DT = mybir.dt

# stats buffer column layout (all f32, each column written exactly once)
N_TAILS = (NTILES - 1) + 2              # 3 full tails + 2 half tails = 5
COL_UALL = 0                            # sum A per tail
COL_B = COL_UALL + N_TAILS              # sum B per tail
COL_POSB = COL_B + N_TAILS              # sum pos*B per tail
COL_POSA = COL_POSB + N_TAILS           # sum pos*A per tail
STAT_COLS = COL_POSA + N_TAILS          # 20
NSTAT = STAT_COLS  # legacy alias


def build_kernel() -> bass.Bass:
    # Bacc (not raw Bass): its compile() pipeline runs
    # generate_event_semaphores, which splits multi-sem waits to satisfy the
    # 1-wait-per-instruction TRN2 sync structs -- raw Bass modules with
    # Tile-emitted multi-waits fail walrus codegen.
    nc = bacc.Bacc("TRN2")

    predict = nc.declare_dram_parameter("predict", [C, PIX], DT.float32, isOutput=False)
    target = nc.declare_dram_parameter("target", [P, FCOLS], DT.bfloat16, isOutput=False)
    idn = nc.declare_dram_parameter("idn", [P, P], DT.bfloat16, isOutput=False)
    out = nc.declare_dram_parameter("out", [P, STAT_COLS], DT.float32, isOutput=True)

    pred_r = predict.rearrange("c (p f) -> c p f", p=P)  # [19, 128, 4096]

    with tile.TileContext(nc) as tc, ExitStack() as ctx:
        const = ctx.enter_context(tc.tile_pool(name="const", bufs=1))
        p_pool = ctx.enter_context(tc.tile_pool(name="p", bufs=16))
        lm_pool = ctx.enter_context(tc.tile_pool(name="lm", bufs=14))
        pix_pool = ctx.enter_context(tc.tile_pool(name="pix", bufs=2))
        scr_pool = ctx.enter_context(tc.tile_pool(name="scr", bufs=2))
        eq_pool = ctx.enter_context(tc.tile_pool(name="eq", bufs=4))
        psum_pool = ctx.enter_context(tc.tile_pool(name="ps", bufs=2, space="PSUM"))

        # constants + target go through the gpsimd queue so the sync queue's
        # first instruction is the first predict load
        idn_sb = const.tile([P, P], DT.bfloat16, tag="idn")
        nc.gpsimd.dma_start(out=idn_sb[:], in_=idn[:])
        t_bf = const.tile([P, FCOLS], DT.bfloat16, tag="tb")
        nc.gpsimd.dma_start(out=t_bf[:], in_=target[:])

        stats = const.tile([P, STAT_COLS], DT.float32, tag="stats")
        neg1 = const.tile([P, 1], DT.float32, tag="neg1")
        nc.gpsimd.memset(neg1[:], -1.0)
        two_ = const.tile([P, 1], DT.float32, tag="two")
        nc.gpsimd.memset(two_[:], 2.0)
        dscr = const.tile([P, 1], DT.float32, tag="dscr")

        def class_pass(p_src, t_sl, w, acc_ps, acc_off, c, last_c):
            """One class over w pixel cols: Ln, eq, mask, PE accumulate.

            lm layout [L(w) | masked(w)]; A contribs go to acc cols
            [acc_off, acc_off+w), Lsel contribs to [FT+acc_off, FT+acc_off+w).
            """
            lm_full = lm_pool.tile([P, 2 * FT], DT.bfloat16, tag="lm", name="lm")
            lm = lm_full[:, : 2 * w]
            nc.scalar.activation(
                out=lm[:, :w],
                in_=p_src,
                func=mybir.ActivationFunctionType.Ln,
                bias=1.0,
                scale=-1.0,
            )
            eq_full = eq_pool.tile([P, FT], DT.bfloat16, tag="eq", name="eq")
            eq = eq_full[:, :w]
            nc.vector.tensor_scalar(
                out=eq[:],
                in0=t_sl,
                scalar1=float(c),
                scalar2=None,
                op0=mybir.AluOpType.is_equal,
            )
            nc.vector.tensor_mul(out=lm[:, w:], in0=eq[:], in1=lm[:, :w])
            nseg = w // 512
            for s in range(2 * nseg):
                ssl = slice(s * 512, (s + 1) * 512)
                if s < nseg:
                    aoff = acc_off + s * 512
                else:
                    aoff = FT + acc_off + (s - nseg) * 512
                nc.tensor.matmul(
                    acc_ps[:, aoff : aoff + 512],
                    lhsT=idn_sb[:],
                    rhs=lm[:, ssl],
                    start=(c == 0),
                    stop=last_c,
                )

        for t in range(NTILES):
            fsl = slice(t * FT, (t + 1) * FT)
            last = t == NTILES - 1
            # PSUM acc: [A(1024) | L_sel(1024)]
            acc_ps = psum_pool.tile([P, 2 * FT], DT.float32, tag="acc")

            for c in range(C):
                p_t = p_pool.tile([P, FT], DT.float32, tag="p")
                if last and c == C - 1:
                    # split the final class: DMA + compute per 512-col half so
                    # the end-of-stream dependency chain operates on halves
                    for h in range(2):
                        hsl = slice(h * HALF, (h + 1) * HALF)
                        gsl = slice(t * FT + h * HALF, t * FT + (h + 1) * HALF)
                        nc.sync.dma_start(out=p_t[:, hsl], in_=pred_r[c, :, gsl])
                        class_pass(
                            p_t[:, hsl], t_bf[:, gsl], HALF, acc_ps, h * HALF, c, True
                        )
                else:
                    # p bufs=8 aligns slot reuse with the global DMA->DMAHW-
                    # proc round-robin (8 procs), so the WAW on the old writer
                    # is same-proc FIFO order and Tile emits no cross-queue wait
                    nc.sync.dma_start(out=p_t[:], in_=pred_r[c, :, fsl])
                    class_pass(p_t[:], t_bf[:, fsl], FT, acc_ps, 0, c, c == C - 1)

            if not last:
                # tail over the full 1024 cols
                k = t
                a_ps = acc_ps[:, :FT]
                lsel_ps = acc_ps[:, FT:]
                t_sl = t_bf[:, fsl]
                nc.vector.tensor_reduce(
                    out=stats[:, COL_UALL + k : COL_UALL + k + 1],
                    in_=a_ps,
                    axis=mybir.AxisListType.X,
                    op=mybir.AluOpType.add,
                )
                scr = scr_pool.tile([P, FT], DT.float32, tag="scr", name="scr")
                nc.vector.scalar_tensor_tensor(
                    out=scr[:],
                    in0=t_sl,
                    scalar=0.5,
                    in1=a_ps,
                    op0=mybir.AluOpType.is_gt,
                    op1=mybir.AluOpType.mult,
                    accum_out=stats[:, COL_POSA + k : COL_POSA + k + 1],
                )
                expl = pix_pool.tile([P, FT], DT.float32, tag="expl", name="expl")
                nc.scalar.activation(
                    out=expl[:],
                    in_=lsel_ps,
                    func=mybir.ActivationFunctionType.Exp,
                    scale=-1.0,
                )
                bq = pix_pool.tile([P, FT], DT.float32, tag="bq", name="bq")
                nc.scalar.activation(
                    out=bq[:],
                    in_=expl[:],
                    func=mybir.ActivationFunctionType.Ln,
                    bias=neg1[:, 0:1],
                )
                nc.vector.tensor_reduce(
                    out=stats[:, COL_B + k : COL_B + k + 1],
                    in_=bq[:],
                    axis=mybir.AxisListType.X,
                    op=mybir.AluOpType.add,
                )
                nc.vector.scalar_tensor_tensor(
                    out=scr[:],
                    in0=t_sl,
                    scalar=0.5,
                    in1=bq[:],
                    op0=mybir.AluOpType.is_gt,
                    op1=mybir.AluOpType.mult,
                    accum_out=stats[:, COL_POSB + k : COL_POSB + k + 1],
                )
            else:
                # last tile: tail in two 512-col halves, ACT phases shared so
                # the Exp/Ln function tables are each loaded once; a dummy
                # 1-col Exp right after the final class Ln pulls the Exp table
                # load off the critical chain (it has no data deps)
                hs = [
                    slice(t * FT + h * HALF, t * FT + (h + 1) * HALF) for h in range(2)
                ]
                a_h = [acc_ps[:, h * HALF : (h + 1) * HALF] for h in range(2)]
                lsel_h = [
                    acc_ps[:, FT + h * HALF : FT + (h + 1) * HALF] for h in range(2)
                ]
                expl_t = pix_pool.tile([P, FT], DT.float32, tag="expl", name="expl")
                bq_t = pix_pool.tile([P, FT], DT.float32, tag="bq", name="bq")
                scr = scr_pool.tile([P, FT], DT.float32, tag="scr", name="scr")
                for h in range(2):
                    nc.vector.tensor_reduce(
                        out=stats[:, COL_UALL + NTILES - 1 + h : COL_UALL + NTILES + h],
                        in_=a_h[h],
                        axis=mybir.AxisListType.X,
                        op=mybir.AluOpType.add,
                    )
                    nc.vector.scalar_tensor_tensor(
                        out=scr[:, h * HALF : (h + 1) * HALF],
                        in0=t_bf[:, hs[h]],
                        scalar=0.5,
                        in1=a_h[h],
                        op0=mybir.AluOpType.is_gt,
                        op1=mybir.AluOpType.mult,
                        accum_out=stats[
                            :, COL_POSA + NTILES - 1 + h : COL_POSA + NTILES + h
                        ],
                    )
                nc.scalar.activation(
                    out=dscr[:],
                    in_=two_[:],
                    func=mybir.ActivationFunctionType.Exp,
                    scale=-1.0,
                )
                for h in range(2):
                    nc.scalar.activation(
                        out=expl_t[:, h * HALF : (h + 1) * HALF],
                        in_=lsel_h[h],
                        func=mybir.ActivationFunctionType.Exp,
                        scale=-1.0,
                    )
                for h in range(2):
                    nc.scalar.activation(
                        out=bq_t[:, h * HALF : (h + 1) * HALF],
                        in_=expl_t[:, h * HALF : (h + 1) * HALF],
                        func=mybir.ActivationFunctionType.Ln,
                        bias=neg1[:, 0:1],
                    )
                for h in range(2):
                    nc.vector.tensor_reduce(
                        out=stats[:, COL_B + NTILES - 1 + h : COL_B + NTILES + h],
                        in_=bq_t[:, h * HALF : (h + 1) * HALF],
                        axis=mybir.AxisListType.X,
                        op=mybir.AluOpType.add,
                    )
                    nc.vector.scalar_tensor_tensor(
                        out=scr[:, h * HALF : (h + 1) * HALF],
                        in0=t_bf[:, hs[h]],
                        scalar=0.5,
                        in1=bq_t[:, h * HALF : (h + 1) * HALF],
                        op0=mybir.AluOpType.is_gt,
                        op1=mybir.AluOpType.mult,
                        accum_out=stats[
                            :, COL_POSB + NTILES - 1 + h : COL_POSB + NTILES + h
                        ],
                    )

        # ship the raw stats tile; the host folds partitions/columns in f64
        nc.sync.dma_start(out=out[:], in_=stats[:])

    if not nc.is_finalized():
        nc.finalize()

    return nc


def combine_stats(stats_list, pos_total: float) -> np.float32:
    """Host-side fold of the per-core [P, STAT_COLS] stats tiles (f64)."""
    u_all = b_sum = pos_b = pos_a = np.float64(0.0)
    for st in stats_list:
        st = np.asarray(st, dtype=np.float64).reshape(P, STAT_COLS)
        u_all += st[:, COL_UALL : COL_UALL + N_TAILS].sum()
        b_sum += st[:, COL_B : COL_B + N_TAILS].sum()
        pos_b += st[:, COL_POSB : COL_POSB + N_TAILS].sum()
        pos_a += st[:, COL_POSA : COL_POSA + N_TAILS].sum()
    tot = np.float64(len(stats_list) * PIX)
    s_all = -(b_sum + u_all)
    s_pos = -(pos_b + pos_a)
    pos = np.float64(pos_total)
    neg = tot - pos
    s_neg = s_all - s_pos
    loss = ((neg / tot) * s_pos + (pos / tot) * s_neg) / (tot * C)
    return np.float32(loss)


def host_pos(target: np.ndarray) -> float:
    return float((np.asarray(target) > 0).sum())


def make_in_maps(predict: np.ndarray, target: np.ndarray):
    import ml_dtypes

    predict = np.ascontiguousarray(predict, dtype=np.float32)
    target_bf = np.ascontiguousarray(
        np.asarray(target, dtype=np.int32).astype(ml_dtypes.bfloat16)
    )
    idn = np.eye(P, dtype=np.float32).astype(ml_dtypes.bfloat16)
    return [
        {
            "predict": predict[k].reshape(C, PIX),
            "target": target_bf[k].reshape(P, FCOLS),
            "idn": idn,
        }
        for k in range(N_CORES)
    ]


_NC_CACHE = None


def kernel(predict: np.ndarray, target: np.ndarray) -> np.ndarray:
    global _NC_CACHE
    if _NC_CACHE is None:
        _NC_CACHE = build_kernel()
    nc = _NC_CACHE

    in_maps = make_in_maps(predict, target)
    res = run_bass_kernel_spmd(nc, in_maps, list(range(N_CORES)))
    return combine_stats(
        [res.results[k]["out"] for k in range(N_CORES)], host_pos(target)
    )
